# revision 1
# baseline (speedup 1.0000x reference)
"""Causal self-attention (B=2,T=2048,C=1024,H=16) on 8 trn2 cores.

Sharding: core = (batch b, head-group g); b = core//4, g = core%4.
Each core computes attention for 4 heads of one batch plus the
row-parallel slice of c_proj; host sums the 4 partial projections per
batch and adds b_proj.
"""

import numpy as np
from contextlib import ExitStack

import concourse.bass as bass
import concourse.mybir as mybir
import concourse.tile as tile
from concourse.bass import ts, ds
from concourse.bass_utils import run_bass_kernel_spmd
from concourse.vector_clock import ScopedClock

# ---------------------------------------------------------------------------
# Workaround: walrus CoreV3 rejects >2 sem waits on one instruction; the
# TileContext exit drain accumulates one wait per outstanding proc.  Split
# them across single-wait sync nops before the drain.
# ---------------------------------------------------------------------------


def _patched_drain_and_barrier(self, tick_clock, wait_clock):
    nc = self.nc
    probe = mybir.InstNoOp(name=nc.get_next_instruction_name(), ins=[], outs=[])
    probe.engine = mybir.EngineType.SP
    wait_clock.add_sem_waits(probe, ScopedClock({None: tick_clock.global_clock}))
    waits = list(probe.sync_info.on_wait) if probe.sync_info else []
    for w in waits:
        n = nc.sync.nop(nofuse=True, hint="drain_wait_split")
        n.ins.sync_info = mybir.SyncInfo(on_wait=[w], on_update=[])
    nc.sync.drain()
    nc.all_engine_barrier()
    assert self.sems is not None
    popped = nc._tile_sem_poison_stack.pop()
    assert popped is self._sem_poison
    nc.clear_and_free_semaphores(list(self.sems.allocated().values()))
    nc.all_engine_barrier()


tile.TileContext._drain_and_barrier = _patched_drain_and_barrier

_DMA_INSTS = (
    mybir.InstCollectiveCompute,
)


def split_excess_waits(nc):
    """walrus CoreV3 encodes at most 1 sem wait per compute instruction
    (2 on EventSemaphore); hoist extras onto same-engine nops."""
    for fn in nc.m.functions:
        for bb in fn.blocks:
            insts = bb.instructions
            new_list = []
            changed = False
            for inst in insts:
                si = inst.sync_info
                cap = 2 if isinstance(inst, mybir.InstEventSemaphore) else 1
                if (
                    si is not None
                    and not isinstance(inst, _DMA_INSTS)
                    and len(si.on_wait) > cap
                ):
                    waits = list(si.on_wait)
                    extra, keep = waits[:-cap], waits[-cap:]
                    for w in extra:
                        nop = mybir.InstNoOp(
                            name=nc.get_next_instruction_name(), ins=[], outs=[]
                        )
                        nop.engine = inst.engine
                        nop.sync_info = mybir.SyncInfo(on_wait=[w], on_update=[])
                        nc.register_instruction(nop)
                        new_list.append(nop)
                    inst.sync_info = mybir.SyncInfo(
                        on_wait=keep, on_update=list(si.on_update)
                    )
                    changed = True
                new_list.append(inst)
            if changed:
                bb.instructions = new_list

# ---------------------------------------------------------------------------

B, T, C, H, HD = 2, 2048, 1024, 16, 64
NCORES, GROUPS = 8, 4
CL = C // GROUPS          # 256 channels (4 heads) per core
HPC = H // GROUPS         # 4 heads per core
F32 = mybir.dt.float32
R32 = mybir.dt.float32r
MM_DT = mybir.dt.float32r

QT = 512                  # q window (free dim of S^T tiles)
NQW = T // QT             # 4 q windows
NKT = T // 128            # 16 k tiles of 128


def _mm(ap):
    return ap.bitcast(MM_DT)


import os
SECTIONS = os.environ.get("KSECTIONS", "full")
REPS = int(os.environ.get("KREPS", "1"))


def build_nc():
    nc = bass.Bass()
    xT = nc.dram_tensor("xT", [C, T], R32, kind="ExternalInput")
    wqkT = nc.dram_tensor("wqkT", [C, 2 * CL], R32, kind="ExternalInput")
    wvT = nc.dram_tensor("wvT", [C, CL], R32, kind="ExternalInput")
    wpT = nc.dram_tensor("wpT", [CL, C], R32, kind="ExternalInput")
    bqk = nc.dram_tensor("bqk", [2 * CL], F32, kind="ExternalInput")
    bvb = nc.dram_tensor("bvb", [128, CL], F32, kind="ExternalInput")
    m0 = nc.dram_tensor("m0", [128, 128], F32, kind="ExternalInput")
    ones = nc.dram_tensor("ones", [128, NKT * HPC], R32, kind="ExternalInput")
    outp = nc.dram_tensor("outp", [T, C], F32, kind="ExternalOutput")

    AF = mybir.ActivationFunctionType
    OP = mybir.AluOpType

    with tile.TileContext(nc) as tc, ExitStack() as ctx:
        persist = ctx.enter_context(tc.tile_pool(name="persist", bufs=1))
        qkvin = ctx.enter_context(tc.tile_pool(name="qkvin", bufs=1))
        work = ctx.enter_context(tc.tile_pool(name="work", bufs=3))
        bcast = ctx.enter_context(tc.tile_pool(name="bcast", bufs=2))
        outsb = ctx.enter_context(tc.tile_pool(name="outsb", bufs=2))
        psS = ctx.enter_context(tc.tile_pool(name="psS", bufs=2, space="PSUM"))
        psY = ctx.enter_context(tc.tile_pool(name="psY", bufs=3, space="PSUM"))
        psD = ctx.enter_context(tc.tile_pool(name="psD", bufs=1, space="PSUM"))

        # persistent tensors
        qkT = persist.tile([128, 4, T], R32)          # o-tiles: q01 q23 k01 k23
        vaug = persist.tile([128, NKT, HPC, HD + 1], R32)
        yT = persist.tile([128, 2, T], R32)           # heads stacked on (part, chunk)
        wp_s = persist.tile([128, 2, C], R32)
        m0_s = persist.tile([128, 128], F32)
        bq_s = persist.tile([128, 4], F32)
        bv_s = persist.tile([128, CL], F32)

        nc.sync.dma_start(out=wp_s, in_=wpT.rearrange("(cc p) o -> p cc o", p=128))
        nc.gpsimd.dma_start(out=m0_s, in_=m0[:, :])
        nc.gpsimd.dma_start(out=bq_s, in_=bqk.rearrange("(o p) -> p o", p=128))
        nc.gpsimd.dma_start(out=bv_s, in_=bvb[:, :])
        nc.gpsimd.dma_start(
            out=vaug[:, :, :, HD:HD + 1],
            in_=ones.rearrange("p (a b) -> p a b", b=HPC).unsqueeze(3),
        )
        ones_f = persist.tile([1, 64], F32)
        nc.gpsimd.dma_start(out=ones_f, in_=ones[0:1, 0:64].bitcast(F32))

        # ---- QKV inputs ----
        xTs = qkvin.tile([128, 8, T], R32)
        wqk_s = qkvin.tile([128, 8, 2 * CL], R32)
        wv_s = qkvin.tile([128, 8, CL], R32)
        xT_r = xT.rearrange("(cc p) t -> p cc t", p=128)
        for cc in range(8):
            nc.sync.dma_start(out=xTs[:, cc, :], in_=xT_r[:, cc, :])
        nc.sync.dma_start(out=wqk_s, in_=wqkT.rearrange("(cc p) o -> p cc o", p=128))
        nc.sync.dma_start(out=wv_s, in_=wvT.rearrange("(cc p) o -> p cc o", p=128))

        # ---- q^T / k^T : out [o, t], W stationary (reused across 2 t-tiles) ----
        for rep in range(REPS):
          for tp in range(2):
              for o in range(4):
                  sc = 0.125 if o < 2 else 1.0
                  ps = psS.tile([128, 2, QT], F32, tag="s")
                  for cc in range(8):
                      for ti in range(2):
                          t = tp * 2 + ti
                          nc.tensor.matmul(
                              ps[:, ti, :],
                              lhsT=_mm(wqk_s[:, cc, ts(o, 128)]),
                              rhs=_mm(xTs[:, cc, ts(t, QT)]),
                              start=(cc == 0),
                              stop=(cc == 7),
                          )
                  for ti in range(2):
                      t = tp * 2 + ti
                      nc.vector.tensor_scalar(
                          out=qkT[:, o, ts(t, QT)],
                          in0=ps[:, ti, :],
                          scalar1=sc,
                          scalar2=bq_s[:, o:o + 1],
                          op0=OP.mult,
                          op1=OP.add,
                      )

          # ---- v : out [t, o] natural layout (+ bias) ----
          for tt in range(NKT):
              psv = psY.tile([128, CL], F32, tag="y")
              for cc in range(8):
                  nc.tensor.matmul(
                      psv,
                      lhsT=_mm(xTs[:, cc, ts(tt, 128)]),
                      rhs=_mm(wv_s[:, cc, :]),
                      start=(cc == 0),
                      stop=(cc == 7),
                  )
              nc.vector.tensor_add(
                  out=vaug[:, tt, :, 0:HD],
                  in0=psv.rearrange("p (h d) -> p h d", h=HPC),
                  in1=bv_s.rearrange("p (h d) -> p h d", h=HPC),
              )

          # ---- attention: per (q-window j, head h) ----
          for j in (range(NQW) if SECTIONS in ("full", "qkvattn") else []):
              nkt = 4 * (j + 1)            # causal: only k tiles <= window end
              for h in range(HPC):
                  hp, w = h // 2, h % 2
                  pl = 64 * w
                  psy = psY.tile([65, QT], F32, tag="y")
                  pending = []
                  for g in range(nkt // 2):
                      pt = work.tile([128, 2, QT], R32, tag="pt")
                      pss = psS.tile([128, 2, QT], F32, tag="s")
                      cur = []
                      for i in range(2):
                          kt = 2 * g + i
                          m = kt - 4 * j
                          q_lo = m * 128 if m >= 0 else 0
                          n = QT - q_lo
                          nc.tensor.matmul(
                              pss[:, i, q_lo:QT],
                              lhsT=_mm(qkT[pl:pl + 64, 2 + hp, ts(kt, 128)]),
                              rhs=_mm(qkT[pl:pl + 64, hp, ds(j * QT + q_lo, n)]),
                              start=True,
                              stop=True,
                          )
                          cur.append((pt, i, kt, q_lo))
                      if cur[0][3] == 0 and cur[1][3] == 0:
                          nc.scalar.activation(out=pt[:, :, :], in_=pss[:, :, :], func=AF.Exp)
                      else:
                          for (ptile, i, kt, q_lo) in cur:
                              nc.scalar.activation(
                                  out=ptile[:, i, q_lo:QT], in_=pss[:, i, q_lo:QT], func=AF.Exp
                              )
                      for (ptile, i, kt, q_lo) in cur:
                          if kt - 4 * j >= 0:
                              nc.vector.tensor_mul(
                                  out=ptile[:, i, ds(q_lo, 128)],
                                  in0=ptile[:, i, ds(q_lo, 128)],
                                  in1=m0_s,
                              )
                      pending.append(cur)
                      if len(pending) > 2:
                          for (ptile, i, kt, q_lo) in pending.pop(0):
                              nc.tensor.matmul(
                                  psy[:, q_lo:QT],
                                  lhsT=_mm(vaug[:, kt, h, :]),
                                  rhs=_mm(ptile[:, i, q_lo:QT]),
                                  start=(kt == 0),
                                  stop=(kt == nkt - 1),
                              )
                  for grp in pending:
                      for (ptile, i, kt, q_lo) in grp:
                          nc.tensor.matmul(
                              psy[:, q_lo:QT],
                              lhsT=_mm(vaug[:, kt, h, :]),
                              rhs=_mm(ptile[:, i, q_lo:QT]),
                              start=(kt == 0),
                              stop=(kt == nkt - 1),
                          )
                  # normalize: y^T = y_aug^T * (1/denom) broadcast over partitions
                  rc = bcast.tile([1, QT], F32, tag="rc")
                  dn = bcast.tile([64, QT], F32, tag="dn")
                  ps_dn = psD.tile([64, QT], F32, tag="dn")
                  nc.vector.reciprocal(out=rc, in_=psy[64:65, :])
                  nc.tensor.matmul(ps_dn, lhsT=ones_f, rhs=rc, start=True, stop=True)
                  nc.scalar.copy(out=dn, in_=ps_dn)
                  nc.vector.tensor_mul(
                      out=yT[pl:pl + 64, hp, ts(j, QT)],
                      in0=psy[0:64, :],
                      in1=dn,
                  )

              # ---- c_proj for the 4 finished t-tiles of this window ----
              for tl in (range(4) if SECTIONS == "full" else []):
                  tt = j * 4 + tl
                  ob = outsb.tile([128, C], F32, tag="ob")
                  for nn_ in range(2):
                      pso = psY.tile([128, 512], F32, tag="y")
                      for c2 in range(2):
                          nc.tensor.matmul(
                              pso,
                              lhsT=_mm(yT[:, c2, ts(tt, 128)]),
                              rhs=_mm(wp_s[:, c2, ts(nn_, 512)]),
                              start=(c2 == 0),
                              stop=(c2 == 1),
                          )
                      nc.vector.tensor_copy(out=ob[:, ts(nn_, 512)], in_=pso)
                  nc.sync.dma_start(out=outp[ts(tt, 128), :], in_=ob)

    split_excess_waits(nc)
    return nc


_NC_CACHE = None


def _get_nc():
    global _NC_CACHE
    if _NC_CACHE is None:
        _NC_CACHE = build_nc()
    return _NC_CACHE


def make_in_maps(x, W_attn, b_attn, W_proj):
    x = np.asarray(x, np.float32)
    W_attn = np.asarray(W_attn, np.float32)
    b_attn = np.asarray(b_attn, np.float32)
    W_proj = np.asarray(W_proj, np.float32)
    m0 = np.triu(np.ones((128, 128), np.float32))  # keep q >= k
    in_maps = []
    for core in range(NCORES):
        b, g = core // GROUPS, core % GROUPS
        qr = slice(g * CL, (g + 1) * CL)
        kr = slice(C + g * CL, C + (g + 1) * CL)
        vr = slice(2 * C + g * CL, 2 * C + (g + 1) * CL)
        wqk = np.concatenate([W_attn[qr], W_attn[kr]], axis=0)      # [512, 1024]
        in_maps.append({
            "xT": np.ascontiguousarray(x[b].T),
            "wqkT": np.ascontiguousarray(wqk.T),
            "wvT": np.ascontiguousarray(W_attn[vr].T),
            "wpT": np.ascontiguousarray(W_proj[:, g * CL:(g + 1) * CL].T),
            "bqk": np.concatenate([b_attn[qr] / 8.0, b_attn[kr]]),
            "bvb": np.broadcast_to(b_attn[vr], (128, CL)).copy(),
            "m0": m0,
            "ones": np.ones((128, 64), np.float32),
        })
    return in_maps


def kernel(x, W_attn, b_attn, W_proj, b_proj, **_unused):
    nc = _get_nc()
    in_maps = make_in_maps(x, W_attn, b_attn, W_proj)
    res = run_bass_kernel_spmd(nc, in_maps, core_ids=list(range(NCORES)))
    out = np.zeros((B, T, C), np.float32)
    for core in range(NCORES):
        out[core // GROUPS] += res.results[core]["outp"]
    out += np.asarray(b_proj, np.float32)[None, None, :]
    return out

